# revision 10
# baseline (speedup 1.0000x reference)
"""GGNN message passing Trainium2 Bass kernel, v2.

Problem (hardcoded, self-contained):
  node_state [32, 1024, 64] f32, adj_mat [32, 1024, 1024] i32 (values 0..3),
  matrix_in/matrix_out [4, 64, 64] f32, bias [128] f32.
  out[b,i,:64]  = sum_j matrix_in [adj[b,i,j]] @ h[b,j] + bias[:64]
  out[b,i,64:]  = sum_j matrix_out[adj[b,j,i]] @ h[b,j] + bias[64:]

Data-parallel over batch: 4 batches per core on 8 cores.

Algorithm (per batch, per direction):
  Host recodes adjacency into one byte-plane B with per-class codes
  {0x3c, 0x30, 0x3e, 0x3d}.  The SAME bytes read as fp8e4m3 and as fp8e5m2
  give two affinely-independent functions of the class (the two formats
  place exponent-binade boundaries differently), so two of the three basis
  planes are just dtype-bitcast views of B — zero on-chip work.  The third
  basis plane is one u16-SIMD bitwise AND (B & 0x3131 isolates class 3,
  whose code is the only one with bit0 set).  With basis
  {1, e4(B), e5(B), e4(B&0x31)} the per-class matrices decompose as
  M[a] = sum_k D_k f_k(a); host sends Q_k = h @ D_k.T as exact fp8 hi/lo
  pairs.  Stage-1 is fp8 DoubleRow matmuls (K=256/instr, 2x rate):
  psum.T[d2, i] = sum_k sum_j plane_k[j, i] * Q_k[j, d2].
  Host-exact corrections for Q quantization plus D0 @ hsum + bias fold into
  one f32 const column added during PSUM evacuation (Act Identity+bias
  stages ps_hi + const to SBUF, DVE adds ps_lo and casts bf16).
  In-direction uses host-transposed planes; outputs leave as m.T in bf16 and
  are transposed/combined on the host.
"""
import sys

sys.path.insert(0, "/opt/trn_rl_repo")

import numpy as np
import ml_dtypes

from concourse import bacc, bass, mybir, tile
from concourse.bass_utils import run_bass_kernel_spmd  # noqa: F401  (kept for harness use)

f8 = ml_dtypes.float8_e4m3
bf16 = ml_dtypes.bfloat16
dt = mybir.dt
Alu = mybir.AluOpType

NCORES = 8
BATCH = 32
BPC = BATCH // NCORES
N = 1024
D = 64
NT = N // 128


def build_program(reps=1):
    nc = bacc.Bacc("TRN2", target_bir_lowering=False, debug=False)

    # dim1 r: 0 = in-direction (transposed planes), 1 = out-direction
    # bp and qq arrive pre-shuffled to the SBUF layout so every DMA
    # descriptor is a long contiguous per-partition run.
    bp_d = nc.dram_tensor(
        "bp", [BPC, 128, 2, NT, N], dt.float8e4, kind="ExternalInput"
    )
    q_d = nc.dram_tensor(
        "qq", [BPC, 128, 2, 3, NT, 128], dt.float8e4, kind="ExternalInput"
    )
    cc_d = nc.dram_tensor("cc", [BPC, 2, D], dt.float32, kind="ExternalInput")
    o_d = nc.dram_tensor("o", [BPC, 2, D, N], dt.bfloat16, kind="ExternalOutput")

    with tile.TileContext(nc) as tc:
        with (
            tc.tile_pool(name="bp", bufs=3) as bp_pool,
            tc.tile_pool(name="pl", bufs=2) as pl_pool,
            tc.tile_pool(name="q", bufs=3) as q_pool,
            tc.tile_pool(name="cc", bufs=1) as cc_pool,
            tc.tile_pool(name="o", bufs=2) as o_pool,
            tc.tile_pool(name="ev", bufs=2) as ev_pool,
            tc.tile_pool(name="ps", bufs=2, space="PSUM") as psA_pool,
            tc.tile_pool(name="ps2", bufs=2, space="PSUM") as psB_pool,
        ):
            cc_t = cc_pool.tile([D, BPC * 2], dt.float32)
            nc.sync.dma_start(cc_t[:], cc_d[:].rearrange("b r d -> d (b r)"))

            for b_ in range(BPC * reps):
                b = b_ % BPC
                bp_t = bp_pool.tile([128, 2, NT, N], dt.float8e4)
                nc.sync.dma_start(bp_t[:, 0], bp_d[b, :, 0])
                nc.scalar.dma_start(bp_t[:, 1], bp_d[b, :, 1])
                q_t = q_pool.tile([128, 2, 3, NT, 128], dt.float8e4)
                nc.sync.dma_start(q_t[:], q_d[b])

                # third basis plane: isolate the class-3 bit (both
                # orientations in one u16-SIMD AND)
                p3_t = pl_pool.tile([128, 2, NT, N], dt.float8e4)
                nc.vector.tensor_scalar(
                    p3_t[:].bitcast(dt.uint16), bp_t[:].bitcast(dt.uint16),
                    0x3131, None, Alu.bitwise_and,
                )

                o_t = o_pool.tile([D, 2, N], dt.bfloat16)
                for r, ps_pool in ((0, psA_pool), (1, psB_pool)):
                    ps = ps_pool.tile([128, N], dt.float32)
                    for half in range(2):
                        sl = slice(half * 512, (half + 1) * 512)
                        for k in range(3):
                            for t in range(NT // 2):
                                if k == 0:
                                    rhs = bp_t[:, r, 2 * t : 2 * t + 2, sl]
                                elif k == 1:
                                    rhs = bp_t[
                                        :, r, 2 * t : 2 * t + 2, sl
                                    ].bitcast(dt.float8e5)
                                else:
                                    rhs = p3_t[:, r, 2 * t : 2 * t + 2, sl]
                                nc.tensor.matmul(
                                    ps[:, sl],
                                    q_t[:, r, k, 2 * t : 2 * t + 2, :],
                                    rhs,
                                    start=(k == 0 and t == 0),
                                    stop=(k == 2 and t == NT // 2 - 1),
                                    perf_mode=mybir.MatmulPerfMode.DoubleRow,
                                )
                    # evac: Act stages (ps_hi + const) to SBUF, DVE adds ps_lo
                    # (only one PSUM operand allowed per instruction)
                    ev_t = ev_pool.tile([D, N], dt.float32, name=f"ev{r}")
                    nc.scalar.activation(
                        ev_t[:], ps[0:D, :],
                        mybir.ActivationFunctionType.Identity,
                        bias=cc_t[:, 2 * b + r : 2 * b + r + 1], scale=1.0,
                    )
                    nc.vector.tensor_tensor(
                        o_t[:, r, :], ev_t[:], ps[D:128, :], Alu.add
                    )
                nc.scalar.dma_start(o_d[b].rearrange("r d i -> d r i"), o_t[:])

    nc.compile()
    return nc


CODE = np.array([0x3C, 0x30, 0x3E, 0x3D], np.uint8)  # per-class byte codes


def host_prep(node_state, adj_mat, matrix_in, matrix_out, bias):
    """Build the per-batch device inputs: bp, qq, cc (full-batch arrays)."""
    import ml_dtypes as mld

    f8e5 = mld.float8_e5m2
    a8 = adj_mat.astype(np.uint8)
    braw = CODE[a8]
    bp = np.empty((BATCH, 2, N, N), np.uint8)
    bp[:, 1] = braw
    bp[:, 0] = braw.transpose(0, 2, 1)
    # [B, 2, (t p), i] -> [B, p, 2, t, i]  (SBUF layout, contiguous DMA)
    bp = np.ascontiguousarray(
        bp.reshape(BATCH, 2, NT, 128, N).transpose(0, 3, 1, 2, 4)
    )

    # basis values per class: ones, e4m3(code), e5m2(code), e4m3(code & 0x31)
    v4c = CODE.view(f8).astype(np.float64)
    v5c = CODE.view(f8e5).astype(np.float64)
    p3c = (CODE & 0x31).view(f8).astype(np.float64)
    basis = np.stack([np.ones(4), v4c, v5c, p3c])  # [4 basis, 4 classes]
    binv = np.linalg.inv(basis)  # M[a] = sum_k D_k basis[k, a]
    fbar = np.array([v4c.mean(), v5c.mean(), p3c.mean()])

    h32 = node_state.astype(np.float32)
    hsum = h32.sum(axis=1, dtype=np.float64)  # [B, 64]

    qq = np.empty((BATCH, 2, 3, N, 128), f8)  # reshuffled to SBUF layout below
    cc = np.empty((BATCH, 2, D), np.float32)
    for r, M in ((0, matrix_in.astype(np.float64)), (1, matrix_out.astype(np.float64))):
        Dk = np.einsum("ade,ak->kde", M, binv)  # [4, d, e]
        const = hsum @ Dk[0].T + bias[r * D : (r + 1) * D].astype(np.float64)
        for k in range(1, 4):
            Qf = np.einsum(
                "bje,de->bjd", h32, Dk[k].astype(np.float32), dtype=np.float32
            )
            hi = Qf.astype(f8)
            lo = (Qf - hi.astype(np.float32)).astype(f8)
            qq[:, r, k - 1, :, 0:D] = hi
            qq[:, r, k - 1, :, D:128] = lo
            qtrue = Qf.sum(axis=1, dtype=np.float64)
            qq_sum = (hi.astype(np.float32) + lo.astype(np.float32)).sum(
                axis=1, dtype=np.float64
            )
            const = const + fbar[k - 1] * (qtrue - qq_sum)
        cc[:, r] = const.astype(np.float32)
    # [B, 2, 3, (t p), d2] -> [B, p, 2, 3, t, d2]  (SBUF layout, contiguous DMA)
    qq_dev = np.ascontiguousarray(
        qq.reshape(BATCH, 2, 3, NT, 128, 128).transpose(0, 4, 1, 2, 3, 5)
    )
    return bp.view(f8), qq_dev, cc


class Runner:
    """Cached jitted SPMD executor for one built program (bass2jax path)."""

    def __init__(self, reps=1):
        import jax
        from jax.sharding import Mesh, PartitionSpec
        from jax.experimental.shard_map import shard_map
        from concourse import bass2jax

        self.jax = jax
        bass2jax.install_neuronx_cc_hook()
        nc = build_program(reps)
        self.nc = nc

        partition_name = (
            nc.partition_id_tensor.name if nc.partition_id_tensor else None
        )
        in_names, out_names, out_avals, zero_outs = [], [], [], []
        for alloc in nc.m.functions[0].allocations:
            if not isinstance(alloc, mybir.MemoryLocationSet):
                continue
            name = alloc.memorylocations[0].name
            if alloc.kind == "ExternalInput":
                if name != partition_name:
                    in_names.append(name)
            elif alloc.kind == "ExternalOutput":
                shape = tuple(alloc.tensor_shape)
                np_dt = mybir.dt.np(alloc.dtype)
                out_names.append(name)
                out_avals.append(jax.core.ShapedArray(shape, np_dt))
                zero_outs.append(np.zeros(shape, np_dt))
        self.in_names, self.out_names = in_names, out_names
        self.out_avals, self.zero_outs = out_avals, zero_outs
        n_params, n_outs = len(in_names), len(out_names)
        donate = tuple(range(n_params, n_params + n_outs))

        bind_names = in_names + out_names
        if partition_name is not None:
            bind_names = bind_names + [partition_name]

        def _body(*args):
            operands = list(args)
            if partition_name is not None:
                operands.append(bass2jax.partition_id_tensor())
            outs = bass2jax._bass_exec_p.bind(
                *operands,
                out_avals=tuple(out_avals),
                in_names=tuple(bind_names),
                out_names=tuple(out_names),
                lowering_input_output_aliases=(),
                sim_require_finite=True,
                sim_require_nnan=True,
                nc=nc,
            )
            return tuple(outs)

        devices = jax.devices()[:NCORES]
        mesh = Mesh(np.asarray(devices), ("core",))
        self.mesh = mesh
        in_specs = (PartitionSpec("core"),) * (n_params + n_outs)
        out_specs = (PartitionSpec("core"),) * n_outs
        self.fn = jax.jit(
            shard_map(
                _body, mesh=mesh, in_specs=in_specs, out_specs=out_specs,
                check_rep=False,
            ),
            donate_argnums=donate,
            keep_unused=True,
        )

    def concat_inputs(self, in_maps):
        return [
            np.concatenate([np.asarray(m[n]) for m in in_maps], axis=0)
            for n in self.in_names
        ]

    def zeros(self):
        return [
            np.zeros((NCORES * z.shape[0], *z.shape[1:]), z.dtype)
            for z in self.zero_outs
        ]

    def __call__(self, concat_in, zeros=None):
        out = self.fn(*concat_in, *(zeros if zeros is not None else self.zeros()))
        return out


_CACHE = {}


def _get_runner(reps=1):
    if reps not in _CACHE:
        _CACHE[reps] = Runner(reps)
    return _CACHE[reps]


def _prep_in_maps(node_state, adj_mat, matrix_in, matrix_out, bias):
    bp, qq, cc = host_prep(node_state, adj_mat, matrix_in, matrix_out, bias)
    in_maps = []
    for c in range(NCORES):
        sl = slice(c * BPC, (c + 1) * BPC)
        in_maps.append(
            {
                "bp": np.ascontiguousarray(bp[sl]),
                "qq": np.ascontiguousarray(qq[sl]),
                "cc": np.ascontiguousarray(cc[sl]),
            }
        )
    return in_maps


def _assemble(out_arrs, out_names, out_avals):
    o_all = np.asarray(out_arrs[out_names.index("o")])
    o_all = o_all.reshape(BATCH, 2, D, N)
    # [B, 2, D, N] -> [B, N, 2D]
    return (
        o_all.transpose(0, 3, 1, 2).reshape(BATCH, N, 2 * D).astype(np.float32)
    )


def kernel(node_state, adj_mat, matrix_in, matrix_out, bias):
    node_state = np.asarray(node_state, np.float32)
    adj_mat = np.asarray(adj_mat, np.int32)
    matrix_in = np.asarray(matrix_in, np.float32)
    matrix_out = np.asarray(matrix_out, np.float32)
    bias = np.asarray(bias, np.float32)

    runner = _get_runner(1)
    in_maps = _prep_in_maps(node_state, adj_mat, matrix_in, matrix_out, bias)
    out_arrs = runner(runner.concat_inputs(in_maps))
    return _assemble(out_arrs, runner.out_names, runner.out_avals)


# revision 11
# speedup vs baseline: 2.1976x; 2.1976x over previous
"""GGNN message passing Trainium2 Bass kernel, v2.

Problem (hardcoded, self-contained):
  node_state [32, 1024, 64] f32, adj_mat [32, 1024, 1024] i32 (values 0..3),
  matrix_in/matrix_out [4, 64, 64] f32, bias [128] f32.
  out[b,i,:64]  = sum_j matrix_in [adj[b,i,j]] @ h[b,j] + bias[:64]
  out[b,i,64:]  = sum_j matrix_out[adj[b,j,i]] @ h[b,j] + bias[64:]

Data-parallel over batch: 4 batches per core on 8 cores.

Algorithm (per batch, per direction):
  Host recodes adjacency into one byte-plane B with per-class codes
  {0x3c, 0x30, 0x3e, 0x3d}.  The SAME bytes read as fp8e4m3 and as fp8e5m2
  give two affinely-independent functions of the class (the two formats
  place exponent-binade boundaries differently), so two of the three basis
  planes are just dtype-bitcast views of B — zero on-chip work.  The third
  basis plane is one u16-SIMD bitwise AND (B & 0x3131 isolates class 3,
  whose code is the only one with bit0 set).  With basis
  {1, e4(B), e5(B), e4(B&0x31)} the per-class matrices decompose as
  M[a] = sum_k D_k f_k(a); host sends Q_k = h @ D_k.T as exact fp8 hi/lo
  pairs.  Stage-1 is fp8 DoubleRow matmuls (K=256/instr, 2x rate):
  psum.T[d2, i] = sum_k sum_j plane_k[j, i] * Q_k[j, d2].
  Host-exact corrections for Q quantization plus D0 @ hsum + bias fold into
  one f32 const column added during PSUM evacuation (Act Identity+bias
  stages ps_hi + const to SBUF, DVE adds ps_lo and casts bf16).
  In-direction uses host-transposed planes; outputs leave as m.T in bf16 and
  are transposed/combined on the host.
"""
import sys

sys.path.insert(0, "/opt/trn_rl_repo")

import numpy as np
import ml_dtypes

from concourse import bacc, bass, mybir, tile
from concourse.bass_utils import run_bass_kernel_spmd  # noqa: F401  (kept for harness use)

f8 = ml_dtypes.float8_e4m3
bf16 = ml_dtypes.bfloat16
dt = mybir.dt
Alu = mybir.AluOpType

NCORES = 8
BATCH = 32
BPC = BATCH // NCORES
N = 1024
D = 64
NT = N // 128


def build_program(reps=1):
    nc = bacc.Bacc("TRN2", target_bir_lowering=False, debug=False)

    # dim1 r: 0 = in-direction (transposed planes), 1 = out-direction
    # bp and qq arrive pre-shuffled to the SBUF layout so every DMA
    # descriptor is a long contiguous per-partition run.
    bp_d = nc.dram_tensor(
        "bp", [BPC, 128, 2, NT, N], dt.float8e4, kind="ExternalInput"
    )
    q_d = nc.dram_tensor(
        "qq", [BPC, 128, 2, 3, NT, 128], dt.float8e4, kind="ExternalInput"
    )
    cc_d = nc.dram_tensor("cc", [BPC, 2, D], dt.float32, kind="ExternalInput")
    o_d = nc.dram_tensor("o", [BPC, 2, D, N], dt.bfloat16, kind="ExternalOutput")

    with tile.TileContext(nc) as tc:
        with (
            tc.tile_pool(name="bp", bufs=3) as bp_pool,
            tc.tile_pool(name="pl", bufs=2) as pl_pool,
            tc.tile_pool(name="q", bufs=3) as q_pool,
            tc.tile_pool(name="cc", bufs=1) as cc_pool,
            tc.tile_pool(name="o", bufs=2) as o_pool,
            tc.tile_pool(name="ev", bufs=2) as ev_pool,
            tc.tile_pool(name="ps", bufs=2, space="PSUM") as psA_pool,
            tc.tile_pool(name="ps2", bufs=2, space="PSUM") as psB_pool,
        ):
            cc_t = cc_pool.tile([D, BPC * 2], dt.float32)
            nc.sync.dma_start(cc_t[:], cc_d[:].rearrange("b r d -> d (b r)"))

            for b_ in range(BPC * reps):
                b = b_ % BPC
                # balance the two HWDGE queues at ~1.5 MB per batch each
                bp_t = bp_pool.tile([128, 2, NT, N], dt.float8e4)
                nc.sync.dma_start(bp_t[:, 0], bp_d[b, :, 0])
                nc.scalar.dma_start(bp_t[:, 1], bp_d[b, :, 1])
                q_t = q_pool.tile([128, 2, 3, NT, 128], dt.float8e4)
                nc.sync.dma_start(q_t[:, :, 2], q_d[b, :, :, 2])
                nc.scalar.dma_start(q_t[:, :, 0:2], q_d[b, :, :, 0:2])

                # third basis plane: isolate the class-3 bit (both
                # orientations in one u16-SIMD AND)
                p3_t = pl_pool.tile([128, 2, NT, N], dt.float8e4)
                nc.vector.tensor_scalar(
                    p3_t[:].bitcast(dt.uint16), bp_t[:].bitcast(dt.uint16),
                    0x3131, None, Alu.bitwise_and,
                )

                o_t = o_pool.tile([D, 2, N], dt.bfloat16)
                for r, ps_pool in ((0, psA_pool), (1, psB_pool)):
                    ps = ps_pool.tile([128, N], dt.float32)
                    for half in range(2):
                        sl = slice(half * 512, (half + 1) * 512)
                        for k in range(3):
                            for t in range(NT // 2):
                                if k == 0:
                                    rhs = bp_t[:, r, 2 * t : 2 * t + 2, sl]
                                elif k == 1:
                                    rhs = bp_t[
                                        :, r, 2 * t : 2 * t + 2, sl
                                    ].bitcast(dt.float8e5)
                                else:
                                    rhs = p3_t[:, r, 2 * t : 2 * t + 2, sl]
                                nc.tensor.matmul(
                                    ps[:, sl],
                                    q_t[:, r, k, 2 * t : 2 * t + 2, :],
                                    rhs,
                                    start=(k == 0 and t == 0),
                                    stop=(k == 2 and t == NT // 2 - 1),
                                    perf_mode=mybir.MatmulPerfMode.DoubleRow,
                                )
                    # evac: Act stages (ps_hi + const) to SBUF, DVE adds ps_lo
                    # (only one PSUM operand allowed per instruction)
                    ev_t = ev_pool.tile([D, N], dt.float32, name=f"ev{r}")
                    nc.scalar.activation(
                        ev_t[:], ps[0:D, :],
                        mybir.ActivationFunctionType.Identity,
                        bias=cc_t[:, 2 * b + r : 2 * b + r + 1], scale=1.0,
                    )
                    nc.vector.tensor_tensor(
                        o_t[:, r, :], ev_t[:], ps[D:128, :], Alu.add
                    )
                nc.sync.dma_start(o_d[b].rearrange("r d i -> d r i"), o_t[:])

    nc.compile()
    return nc


CODE = np.array([0x3C, 0x30, 0x3E, 0x3D], np.uint8)  # per-class byte codes


def host_prep(node_state, adj_mat, matrix_in, matrix_out, bias):
    """Build the per-batch device inputs: bp, qq, cc (full-batch arrays)."""
    import ml_dtypes as mld

    f8e5 = mld.float8_e5m2
    a8 = adj_mat.astype(np.uint8)
    braw = CODE[a8]
    bp = np.empty((BATCH, 2, N, N), np.uint8)
    bp[:, 1] = braw
    bp[:, 0] = braw.transpose(0, 2, 1)
    # [B, 2, (t p), i] -> [B, p, 2, t, i]  (SBUF layout, contiguous DMA)
    bp = np.ascontiguousarray(
        bp.reshape(BATCH, 2, NT, 128, N).transpose(0, 3, 1, 2, 4)
    )

    # basis values per class: ones, e4m3(code), e5m2(code), e4m3(code & 0x31)
    v4c = CODE.view(f8).astype(np.float64)
    v5c = CODE.view(f8e5).astype(np.float64)
    p3c = (CODE & 0x31).view(f8).astype(np.float64)
    basis = np.stack([np.ones(4), v4c, v5c, p3c])  # [4 basis, 4 classes]
    binv = np.linalg.inv(basis)  # M[a] = sum_k D_k basis[k, a]
    fbar = np.array([v4c.mean(), v5c.mean(), p3c.mean()])

    h32 = node_state.astype(np.float32)
    hsum = h32.sum(axis=1, dtype=np.float64)  # [B, 64]

    qq = np.empty((BATCH, 2, 3, N, 128), f8)  # reshuffled to SBUF layout below
    cc = np.empty((BATCH, 2, D), np.float32)
    for r, M in ((0, matrix_in.astype(np.float64)), (1, matrix_out.astype(np.float64))):
        Dk = np.einsum("ade,ak->kde", M, binv)  # [4, d, e]
        const = hsum @ Dk[0].T + bias[r * D : (r + 1) * D].astype(np.float64)
        for k in range(1, 4):
            Qf = np.einsum(
                "bje,de->bjd", h32, Dk[k].astype(np.float32), dtype=np.float32
            )
            hi = Qf.astype(f8)
            lo = (Qf - hi.astype(np.float32)).astype(f8)
            qq[:, r, k - 1, :, 0:D] = hi
            qq[:, r, k - 1, :, D:128] = lo
            qtrue = Qf.sum(axis=1, dtype=np.float64)
            qq_sum = (hi.astype(np.float32) + lo.astype(np.float32)).sum(
                axis=1, dtype=np.float64
            )
            const = const + fbar[k - 1] * (qtrue - qq_sum)
        cc[:, r] = const.astype(np.float32)
    # [B, 2, 3, (t p), d2] -> [B, p, 2, 3, t, d2]  (SBUF layout, contiguous DMA)
    qq_dev = np.ascontiguousarray(
        qq.reshape(BATCH, 2, 3, NT, 128, 128).transpose(0, 4, 1, 2, 3, 5)
    )
    return bp.view(f8), qq_dev, cc


class Runner:
    """Cached jitted SPMD executor for one built program (bass2jax path)."""

    def __init__(self, reps=1):
        import jax
        from jax.sharding import Mesh, PartitionSpec
        from jax.experimental.shard_map import shard_map
        from concourse import bass2jax

        self.jax = jax
        bass2jax.install_neuronx_cc_hook()
        nc = build_program(reps)
        self.nc = nc

        partition_name = (
            nc.partition_id_tensor.name if nc.partition_id_tensor else None
        )
        in_names, out_names, out_avals, zero_outs = [], [], [], []
        for alloc in nc.m.functions[0].allocations:
            if not isinstance(alloc, mybir.MemoryLocationSet):
                continue
            name = alloc.memorylocations[0].name
            if alloc.kind == "ExternalInput":
                if name != partition_name:
                    in_names.append(name)
            elif alloc.kind == "ExternalOutput":
                shape = tuple(alloc.tensor_shape)
                np_dt = mybir.dt.np(alloc.dtype)
                out_names.append(name)
                out_avals.append(jax.core.ShapedArray(shape, np_dt))
                zero_outs.append(np.zeros(shape, np_dt))
        self.in_names, self.out_names = in_names, out_names
        self.out_avals, self.zero_outs = out_avals, zero_outs
        n_params, n_outs = len(in_names), len(out_names)
        donate = tuple(range(n_params, n_params + n_outs))

        bind_names = in_names + out_names
        if partition_name is not None:
            bind_names = bind_names + [partition_name]

        def _body(*args):
            operands = list(args)
            if partition_name is not None:
                operands.append(bass2jax.partition_id_tensor())
            outs = bass2jax._bass_exec_p.bind(
                *operands,
                out_avals=tuple(out_avals),
                in_names=tuple(bind_names),
                out_names=tuple(out_names),
                lowering_input_output_aliases=(),
                sim_require_finite=True,
                sim_require_nnan=True,
                nc=nc,
            )
            return tuple(outs)

        devices = jax.devices()[:NCORES]
        mesh = Mesh(np.asarray(devices), ("core",))
        self.mesh = mesh
        in_specs = (PartitionSpec("core"),) * (n_params + n_outs)
        out_specs = (PartitionSpec("core"),) * n_outs
        self.fn = jax.jit(
            shard_map(
                _body, mesh=mesh, in_specs=in_specs, out_specs=out_specs,
                check_rep=False,
            ),
            donate_argnums=donate,
            keep_unused=True,
        )

    def concat_inputs(self, in_maps):
        return [
            np.concatenate([np.asarray(m[n]) for m in in_maps], axis=0)
            for n in self.in_names
        ]

    def zeros(self):
        return [
            np.zeros((NCORES * z.shape[0], *z.shape[1:]), z.dtype)
            for z in self.zero_outs
        ]

    def __call__(self, concat_in, zeros=None):
        out = self.fn(*concat_in, *(zeros if zeros is not None else self.zeros()))
        return out


_CACHE = {}


def _get_runner(reps=1):
    if reps not in _CACHE:
        _CACHE[reps] = Runner(reps)
    return _CACHE[reps]


def _prep_in_maps(node_state, adj_mat, matrix_in, matrix_out, bias):
    bp, qq, cc = host_prep(node_state, adj_mat, matrix_in, matrix_out, bias)
    in_maps = []
    for c in range(NCORES):
        sl = slice(c * BPC, (c + 1) * BPC)
        in_maps.append(
            {
                "bp": np.ascontiguousarray(bp[sl]),
                "qq": np.ascontiguousarray(qq[sl]),
                "cc": np.ascontiguousarray(cc[sl]),
            }
        )
    return in_maps


def _assemble(out_arrs, out_names, out_avals):
    o_all = np.asarray(out_arrs[out_names.index("o")])
    o_all = o_all.reshape(BATCH, 2, D, N)
    # [B, 2, D, N] -> [B, N, 2D]
    return (
        o_all.transpose(0, 3, 1, 2).reshape(BATCH, N, 2 * D).astype(np.float32)
    )


def kernel(node_state, adj_mat, matrix_in, matrix_out, bias):
    node_state = np.asarray(node_state, np.float32)
    adj_mat = np.asarray(adj_mat, np.int32)
    matrix_in = np.asarray(matrix_in, np.float32)
    matrix_out = np.asarray(matrix_out, np.float32)
    bias = np.asarray(bias, np.float32)

    runner = _get_runner(1)
    in_maps = _prep_in_maps(node_state, adj_mat, matrix_in, matrix_out, bias)
    out_arrs = runner(runner.concat_inputs(in_maps))
    return _assemble(out_arrs, runner.out_names, runner.out_avals)


# revision 18
# speedup vs baseline: 3.0658x; 1.3951x over previous
"""GGNN message passing Trainium2 Bass kernel, v2.

Problem (hardcoded, self-contained):
  node_state [32, 1024, 64] f32, adj_mat [32, 1024, 1024] i32 (values 0..3),
  matrix_in/matrix_out [4, 64, 64] f32, bias [128] f32.
  out[b,i,:64]  = sum_j matrix_in [adj[b,i,j]] @ h[b,j] + bias[:64]
  out[b,i,64:]  = sum_j matrix_out[adj[b,j,i]] @ h[b,j] + bias[64:]

Data-parallel over batch: 4 batches per core on 8 cores.

Algorithm (per batch, per direction):
  Host recodes adjacency into one byte-plane B with per-class codes
  {0x3c, 0x30, 0x3e, 0x3d}.  The SAME bytes read as fp8e4m3 and as fp8e5m2
  give two affinely-independent functions of the class (the two formats
  place exponent-binade boundaries differently), so two of the three basis
  planes are just dtype-bitcast views of B — zero on-chip work.  The third
  basis plane is one u16-SIMD bitwise AND (B & 0x3131 isolates class 3,
  whose code is the only one with bit0 set).  With basis
  {1, e4(B), e5(B), e4(B&0x31)} the per-class matrices decompose as
  M[a] = sum_k D_k f_k(a); host sends Q_k = h @ D_k.T as exact fp8 hi/lo
  pairs.  Stage-1 is fp8 DoubleRow matmuls (K=256/instr, 2x rate):
  psum.T[d2, i] = sum_k sum_j plane_k[j, i] * Q_k[j, d2].
  Host-exact corrections for Q quantization plus D0 @ hsum + bias fold into
  one f32 const column added during PSUM evacuation (Act Identity+bias
  stages ps_hi + const to SBUF, DVE adds ps_lo and casts bf16).
  In-direction uses host-transposed planes; outputs leave as m.T in bf16 and
  are transposed/combined on the host.
"""
import sys

sys.path.insert(0, "/opt/trn_rl_repo")

import numpy as np
import ml_dtypes

from concourse import bacc, bass, mybir, tile
from concourse.bass_utils import run_bass_kernel_spmd  # noqa: F401  (kept for harness use)

f8 = ml_dtypes.float8_e4m3
bf16 = ml_dtypes.bfloat16
dt = mybir.dt
Alu = mybir.AluOpType

NCORES = 8
BATCH = 32
BPC = BATCH // NCORES
N = 1024
D = 64
NT = N // 128


def build_program(reps=1):
    nc = bacc.Bacc("TRN2", target_bir_lowering=False, debug=False)

    # dim1 r: 0 = in-direction (transposed planes), 1 = out-direction
    # bp and qq arrive pre-shuffled to the SBUF layout so every DMA
    # descriptor is a long contiguous per-partition run.
    bp_d = nc.dram_tensor(
        "bp", [BPC, 128, 2, NT, N], dt.float8e4, kind="ExternalInput"
    )
    q_d = nc.dram_tensor(
        "qq", [BPC, 128, 2, 3, NT, 128], dt.float8e4, kind="ExternalInput"
    )
    cc_d = nc.dram_tensor("cc", [BPC, 2, D], dt.float32, kind="ExternalInput")
    # [(dir d), i]: both directions stacked on partitions for a full-width DMA
    o_d = nc.dram_tensor("o", [BPC, 2 * D, N], dt.bfloat16, kind="ExternalOutput")

    with tile.TileContext(nc) as tc:
        with (
            tc.tile_pool(name="bp", bufs=4) as bp_pool,
            tc.tile_pool(name="pl", bufs=3) as pl_pool,
            tc.tile_pool(name="q", bufs=4) as q_pool,
            tc.tile_pool(name="cc", bufs=1) as cc_pool,
            tc.tile_pool(name="o", bufs=3) as o_pool,
            tc.tile_pool(name="ev", bufs=4) as ev_pool,
            tc.tile_pool(name="ps", bufs=2, space="PSUM") as psA_pool,
            tc.tile_pool(name="ps2", bufs=2, space="PSUM") as psB_pool,
        ):
            cc_t = cc_pool.tile([D, BPC * 2], dt.float32)
            nc.sync.dma_start(cc_t[:], cc_d[:].rearrange("b r d -> d (b r)"))

            for b_ in range(BPC * reps):
                b = b_ % BPC
                # balance the two HWDGE queues at ~1.5 MB per batch each
                bp_t = bp_pool.tile([128, 2, NT, N], dt.float8e4)
                nc.sync.dma_start(bp_t[:, 0], bp_d[b, :, 0])
                nc.scalar.dma_start(bp_t[:, 1], bp_d[b, :, 1])
                q_t = q_pool.tile([128, 2, 3, NT, 128], dt.float8e4)
                nc.sync.dma_start(q_t[:, :, 2], q_d[b, :, :, 2])
                nc.scalar.dma_start(q_t[:, :, 0:2], q_d[b, :, :, 0:2])

                # third basis plane: isolate the class-3 bit (u16-SIMD AND,
                # one op per orientation so each gates only on its own DMA)
                p3_t = pl_pool.tile([128, 2, NT, N], dt.float8e4)
                for r in range(2):
                    nc.vector.tensor_scalar(
                        p3_t[:, r].bitcast(dt.uint16),
                        bp_t[:, r].bitcast(dt.uint16),
                        0x3131, None, Alu.bitwise_and,
                    )

                o_t = o_pool.tile([2 * D, N], dt.bfloat16)
                for r, ps_pool in ((0, psA_pool), (1, psB_pool)):
                    ps = ps_pool.tile([128, N], dt.float32, name="psd")
                    for half in range(2):
                        sl = slice(half * 512, (half + 1) * 512)
                        for k in range(3):
                            for t in range(NT // 2):
                                if k == 0:
                                    rhs = bp_t[:, r, 2 * t : 2 * t + 2, sl]
                                elif k == 1:
                                    rhs = bp_t[
                                        :, r, 2 * t : 2 * t + 2, sl
                                    ].bitcast(dt.float8e5)
                                else:
                                    rhs = p3_t[:, r, 2 * t : 2 * t + 2, sl]
                                nc.tensor.matmul(
                                    ps[:, sl],
                                    q_t[:, r, k, 2 * t : 2 * t + 2, :],
                                    rhs,
                                    start=(k == 0 and t == 0),
                                    stop=(k == 2 and t == NT // 2 - 1),
                                    perf_mode=mybir.MatmulPerfMode.DoubleRow,
                                )
                    # evac: Act stages (ps_hi + const) to SBUF, DVE adds
                    # ps_lo (only one PSUM operand allowed per instruction)
                    ev_t = ev_pool.tile([D, N], dt.float32, name=f"ev{r}")
                    nc.scalar.activation(
                        ev_t[:], ps[0:D, :],
                        mybir.ActivationFunctionType.Identity,
                        bias=cc_t[:, 2 * b + r : 2 * b + r + 1], scale=1.0,
                    )
                    nc.vector.tensor_tensor(
                        o_t[r * D : (r + 1) * D, :], ev_t[:],
                        ps[D:128, :], Alu.add,
                    )
                # out goes on the gpsimd SWDGE queue: it waits on the evac,
                # and on SP/Act it would head-of-line block the next batch's
                # input DMAs
                nc.gpsimd.dma_start(o_d[b], o_t[:])

    nc.compile()
    return nc


CODE = np.array([0x3C, 0x30, 0x3E, 0x3D], np.uint8)  # per-class byte codes


def host_prep(node_state, adj_mat, matrix_in, matrix_out, bias):
    """Build the per-batch device inputs: bp, qq, cc (full-batch arrays)."""
    import ml_dtypes as mld

    f8e5 = mld.float8_e5m2
    a8 = adj_mat.astype(np.uint8)
    braw = CODE[a8]
    bp = np.empty((BATCH, 2, N, N), np.uint8)
    bp[:, 1] = braw
    bp[:, 0] = braw.transpose(0, 2, 1)
    # [B, 2, (t p), i] -> [B, p, 2, t, i]  (SBUF layout, contiguous DMA)
    bp = np.ascontiguousarray(
        bp.reshape(BATCH, 2, NT, 128, N).transpose(0, 3, 1, 2, 4)
    )

    # basis values per class: ones, e4m3(code), e5m2(code), e4m3(code & 0x31)
    v4c = CODE.view(f8).astype(np.float64)
    v5c = CODE.view(f8e5).astype(np.float64)
    p3c = (CODE & 0x31).view(f8).astype(np.float64)
    basis = np.stack([np.ones(4), v4c, v5c, p3c])  # [4 basis, 4 classes]
    binv = np.linalg.inv(basis)  # M[a] = sum_k D_k basis[k, a]
    fbar = np.array([v4c.mean(), v5c.mean(), p3c.mean()])

    h32 = node_state.astype(np.float32)
    hsum = h32.sum(axis=1, dtype=np.float64)  # [B, 64]

    qq = np.empty((BATCH, 2, 3, N, 128), f8)  # reshuffled to SBUF layout below
    cc = np.empty((BATCH, 2, D), np.float32)
    for r, M in ((0, matrix_in.astype(np.float64)), (1, matrix_out.astype(np.float64))):
        Dk = np.einsum("ade,ak->kde", M, binv)  # [4, d, e]
        const = hsum @ Dk[0].T + bias[r * D : (r + 1) * D].astype(np.float64)
        for k in range(1, 4):
            Qf = np.einsum(
                "bje,de->bjd", h32, Dk[k].astype(np.float32), dtype=np.float32
            )
            hi = Qf.astype(f8)
            lo = (Qf - hi.astype(np.float32)).astype(f8)
            qq[:, r, k - 1, :, 0:D] = hi
            qq[:, r, k - 1, :, D:128] = lo
            qtrue = Qf.sum(axis=1, dtype=np.float64)
            qq_sum = (hi.astype(np.float32) + lo.astype(np.float32)).sum(
                axis=1, dtype=np.float64
            )
            const = const + fbar[k - 1] * (qtrue - qq_sum)
        cc[:, r] = const.astype(np.float32)
    # [B, 2, 3, (t p), d2] -> [B, p, 2, 3, t, d2]  (SBUF layout, contiguous DMA)
    qq_dev = np.ascontiguousarray(
        qq.reshape(BATCH, 2, 3, NT, 128, 128).transpose(0, 4, 1, 2, 3, 5)
    )
    return bp.view(f8), qq_dev, cc


class Runner:
    """Cached jitted SPMD executor for one built program (bass2jax path)."""

    def __init__(self, reps=1):
        import jax
        from jax.sharding import Mesh, PartitionSpec
        from jax.experimental.shard_map import shard_map
        from concourse import bass2jax

        self.jax = jax
        bass2jax.install_neuronx_cc_hook()
        nc = build_program(reps)
        self.nc = nc

        partition_name = (
            nc.partition_id_tensor.name if nc.partition_id_tensor else None
        )
        in_names, out_names, out_avals, zero_outs = [], [], [], []
        for alloc in nc.m.functions[0].allocations:
            if not isinstance(alloc, mybir.MemoryLocationSet):
                continue
            name = alloc.memorylocations[0].name
            if alloc.kind == "ExternalInput":
                if name != partition_name:
                    in_names.append(name)
            elif alloc.kind == "ExternalOutput":
                shape = tuple(alloc.tensor_shape)
                np_dt = mybir.dt.np(alloc.dtype)
                out_names.append(name)
                out_avals.append(jax.core.ShapedArray(shape, np_dt))
                zero_outs.append(np.zeros(shape, np_dt))
        self.in_names, self.out_names = in_names, out_names
        self.out_avals, self.zero_outs = out_avals, zero_outs
        n_params, n_outs = len(in_names), len(out_names)
        donate = tuple(range(n_params, n_params + n_outs))

        bind_names = in_names + out_names
        if partition_name is not None:
            bind_names = bind_names + [partition_name]

        def _body(*args):
            operands = list(args)
            if partition_name is not None:
                operands.append(bass2jax.partition_id_tensor())
            outs = bass2jax._bass_exec_p.bind(
                *operands,
                out_avals=tuple(out_avals),
                in_names=tuple(bind_names),
                out_names=tuple(out_names),
                lowering_input_output_aliases=(),
                sim_require_finite=True,
                sim_require_nnan=True,
                nc=nc,
            )
            return tuple(outs)

        devices = jax.devices()[:NCORES]
        mesh = Mesh(np.asarray(devices), ("core",))
        self.mesh = mesh
        in_specs = (PartitionSpec("core"),) * (n_params + n_outs)
        out_specs = (PartitionSpec("core"),) * n_outs
        self.fn = jax.jit(
            shard_map(
                _body, mesh=mesh, in_specs=in_specs, out_specs=out_specs,
                check_rep=False,
            ),
            donate_argnums=donate,
            keep_unused=True,
        )

    def concat_inputs(self, in_maps):
        return [
            np.concatenate([np.asarray(m[n]) for m in in_maps], axis=0)
            for n in self.in_names
        ]

    def zeros(self):
        return [
            np.zeros((NCORES * z.shape[0], *z.shape[1:]), z.dtype)
            for z in self.zero_outs
        ]

    def __call__(self, concat_in, zeros=None):
        out = self.fn(*concat_in, *(zeros if zeros is not None else self.zeros()))
        return out


_CACHE = {}


def _get_runner(reps=1):
    if reps not in _CACHE:
        _CACHE[reps] = Runner(reps)
    return _CACHE[reps]


def _prep_in_maps(node_state, adj_mat, matrix_in, matrix_out, bias):
    bp, qq, cc = host_prep(node_state, adj_mat, matrix_in, matrix_out, bias)
    in_maps = []
    for c in range(NCORES):
        sl = slice(c * BPC, (c + 1) * BPC)
        in_maps.append(
            {
                "bp": np.ascontiguousarray(bp[sl]),
                "qq": np.ascontiguousarray(qq[sl]),
                "cc": np.ascontiguousarray(cc[sl]),
            }
        )
    return in_maps


def _assemble(out_arrs, out_names, out_avals):
    o_all = np.asarray(out_arrs[out_names.index("o")])
    o_all = o_all.reshape(BATCH, 2, D, N)
    # [B, (2 D), N] -> [B, N, 2D]
    return (
        o_all.transpose(0, 3, 1, 2).reshape(BATCH, N, 2 * D).astype(np.float32)
    )


def kernel(node_state, adj_mat, matrix_in, matrix_out, bias):
    node_state = np.asarray(node_state, np.float32)
    adj_mat = np.asarray(adj_mat, np.int32)
    matrix_in = np.asarray(matrix_in, np.float32)
    matrix_out = np.asarray(matrix_out, np.float32)
    bias = np.asarray(bias, np.float32)

    runner = _get_runner(1)
    in_maps = _prep_in_maps(node_state, adj_mat, matrix_in, matrix_out, bias)
    out_arrs = runner(runner.concat_inputs(in_maps))
    return _assemble(out_arrs, runner.out_names, runner.out_avals)
